# revision 3
# baseline (speedup 1.0000x reference)
"""Trainium2 Bass kernel for CE-with-importance-ratio loss.

Reference computation (B=1, T=2048, V=128256):
    logp = log_softmax(logits.f32, axis=-1)
    sel  = logp[t, labels[t]]
    loss = -sel                 (0 where label == -100)
    ratio = exp(sel - ref_logprobs)   (1 where ignored)
    out = sum(loss * ratio) / count_valid

Sharding: token-parallel across 8 NeuronCores (256 tokens/core).
Each core streams its [256, 128256] bf16 logit shard once from HBM
(tokens on partitions, vocab on the free axis) and emits ONLY the
per-token sum(exp(x)) accumulator columns ([128, ~45] f32); all O(T)
finishing math (label-logit gather, ln, importance ratio, masking,
reduction) runs on the host.

The kernel is DMA-bound.  Trace analysis of the uniform-[128,*]-tile
version showed SDMA engine 15 runs ~15% slower per byte than engines
0-14 (uniformly across packet sizes), and because every [128, w] tile
hands each engine an equal 8-line share, the whole stream was paced by
engine 15 (99.4% busy; others 85%): 195.5us vs the ~183.5us HBM-per-NC
floor (65.7MB @ ~358GB/s).  Engine k serves fixed SBUF partitions
(E15 <-> {92-95, 124-127}), so this version sheds load from E15 by
carving the LAST 8016 vocab columns out of the [128, *] main stream
and re-routing them with rectangular row-splits:

  X = rows  0..91  x cols [V-8016, V) -> [92, 8016] tile, partitions
      0-91 (no E15 lines; ACT-exact exp, own accum column)
  Y = rows 92..127 x cols [V-8016, V) -> [36, 8016] tile placed at
      partition base 0 (engines E0/E2..E14 only; DVE-Schraudolph, two
      accum columns; host maps row p -> token 92+p)

Per block E15 then carries 8 x 120240 x 2B = 1.924MB (~91.2us at its
observed 21.1GB/s) ~= the HBM-floor half-stream, while E0 (the most
loaded fast engine, +8 lines from Y) stays under 89us.  The two
compute engines split the per-block vocab sweep with ~30% slack each:

  ScalarE: exact exp+accumulate on 64128 main cols (tiles of <=16032)
           + the [92, 8016] X tile.
  VectorE: Schraudolph bit-trick exp on 56112 main cols (28 halves of
           2004, paired into one accumulate per 4008) + the [36, 8016]
           Y tile (4 halves, 2 pairs):  i16 = trunc(x * 2^7*log2e + B)
           via one tensor_scalar (bf16 in, 2x mode), then one
           scalar_tensor_tensor that bitcasts two 2004-col halves to
           bf16, adds them, and free-dim-accumulates.  B is
           bias-calibrated for bf16 N(0,1) logits (residual bias ~1e-5;
           final loss error ~1e-6 after averaging).

Block 1's stream ends with small ACT tiles (8016/4008/2004/2004)
interleaved with DVE halves so the compute tail after the final DMA
byte is ~2-3us.  No Ln on device, no indirect gather, no matmul.
"""

import numpy as np

P = 128
B, T, V = 1, 2048, 128256
N_CORES = 8
TS = T // N_CORES          # tokens per core (256)
NB = TS // P               # token blocks per core (2)
IGNORE_INDEX = -100

AT = 16032                 # ScalarE main tile width
DH = 2004                  # VectorE half width (pairs share one accumulate)
D2 = 8016                  # offloaded tail width (cols [V-D2, V))
XR = 92                    # X offload rows (tokens 0..91)
YR = 36                    # Y offload rows (tokens 92..127)
VM = V - D2                # main-stream width (full 128-row tiles)

# Schraudolph exp, 16-bit flavor: i16 = trunc(x * A + B); bitcast the
# i16 to bf16 ~= exp(x).  A = 2^7 * log2(e).  B = 127*2^7 minus a
# bias-correction calibrated on bf16-rounded N(0,1) samples; the DVE's
# f32->i16 convert rounds to nearest, so the truncation-calibrated
# 16249.1316 is shifted down by 0.5.
_A_CONST = 184.6649652337873
_B_CONST = 16248.6316

# Per-block stream layout: (kind, width) in DMA issue order.
#   "A" = [128, w] main ACT tile      "D" = [128, 2004] DVE half
#   "X" = [92, 8016] ACT offload      "Y" = [36, 8016] DVE offload
# X/Y sit at even-D boundaries so the main DVE pairing is undisturbed.
_B0_LAYOUT = (
    [("A", AT)] + [("D", DH)] * 8 +
    [("A", AT), ("X", D2)] + [("D", DH)] * 8 +
    [("A", AT), ("Y", D2)] + [("D", DH)] * 8 +
    [("A", AT)] + [("D", DH)] * 4
)
# Final block: tail interleaves small ACT tiles with DVE halves so
# each engine drains within ~2-3us of the final DMA byte.
_B1_LAYOUT = (
    [("A", AT)] + [("D", DH)] * 8 +
    [("A", AT), ("X", D2)] + [("D", DH)] * 8 +
    [("A", AT), ("Y", D2)] + [("D", DH)] * 6 +
    [("A", 8016)] + [("D", DH)] * 3 +
    [("A", 4008)] + [("D", DH)] * 2 +
    [("A", 2004)] + [("D", DH)] * 1 +
    [("A", 2004)]
)
_LAYOUTS = [_B0_LAYOUT, _B1_LAYOUT]
for _l in _LAYOUTS:
    assert sum(w for k, w in _l if k in ("A", "D")) == VM
    assert sum(1 for k, _ in _l if k == "D") % 2 == 0
    assert sum(1 for k, _ in _l if k == "X") == 1
    assert sum(1 for k, _ in _l if k == "Y") == 1

# Accumulator columns per block, in emission order: one per A tile or
# X tile, one per D pair, two per Y tile.  Host needs (kind, rows).
def _block_cols(layout):
    cols = []
    dpend = 0
    for k, _ in layout:
        if k == "A":
            cols.append(("F", P))
        elif k == "X":
            cols.append(("X", XR))
        elif k == "Y":
            cols.append(("Y", YR))
            cols.append(("Y", YR))
        else:
            dpend += 1
            if dpend == 2:
                cols.append(("F", P))
                dpend = 0
    return cols

_BLK_COLS = [_block_cols(_l) for _l in _LAYOUTS]
_BLK_NCOLS = [len(c) for c in _BLK_COLS]
_NCOLS = sum(_BLK_NCOLS)

_PROGRAM = None


def _build_program():
    import concourse.bacc as bacc
    import concourse.mybir as mybir
    import concourse.tile as tile

    f32 = mybir.dt.float32
    bf16 = mybir.dt.bfloat16
    i16 = mybir.dt.int16

    nc = bacc.Bacc("TRN2", target_bir_lowering=False, debug=False,
                   num_devices=N_CORES)

    logits = nc.dram_tensor("logits", [TS, V], bf16, kind="ExternalInput").ap()
    # Raw accumulator columns per block; the host does the column sum.
    # Block 0's half is DMA'd out mid-kernel (hidden under the stream),
    # so the post-stream critical path is just accum-read -> out issue.
    out = nc.dram_tensor("out", [P, _NCOLS], f32, kind="ExternalOutput").ap()

    Exp = mybir.ActivationFunctionType.Exp
    Add, Mul = mybir.AluOpType.add, mybir.AluOpType.mult

    with tile.TileContext(nc) as tc:
        with (
            tc.tile_pool(name="small", bufs=1) as small,
            tc.tile_pool(name="act", bufs=3) as actp,
            tc.tile_pool(name="dvein", bufs=10) as dvein,
            tc.tile_pool(name="dvet", bufs=4) as dvet,
            tc.tile_pool(name="dveval", bufs=2) as dveval,
            tc.tile_pool(name="xy", bufs=2) as xyp,
        ):
            acc = small.tile([P, _NCOLS], f32)
            # X/Y accum columns only cover rows [0,92)/[0,36); zero the
            # tile once so the out DMA never reads uninitialized SBUF.
            nc.gpsimd.memset(acc[:], 0.0)

            col = 0
            for b, layout in enumerate(_LAYOUTS):
                c0 = col
                off = 0
                ei_prev = None

                def dve_half(x_ap, rows, col):
                    # x_ap: [rows, DH] bf16 already in SBUF
                    nonlocal ei_prev
                    ei = dvet.tile([rows, DH], i16, tag="ei")
                    nc.vector.tensor_scalar(
                        ei[:], x_ap, _A_CONST, _B_CONST, Mul, Add)
                    if ei_prev is None:
                        ei_prev = ei
                        return False
                    val = dveval.tile([rows, DH], bf16, tag="val")
                    nc.vector.scalar_tensor_tensor(
                        val[:], ei_prev[:rows, :].bitcast(bf16), 1.0,
                        ei[:].bitcast(bf16), Mul, Add,
                        accum_out=acc[0:rows, col:col + 1])
                    ei_prev = None
                    return True

                for kind, w in layout:
                    if kind == "A":
                        src = logits[b * P:(b + 1) * P, off:off + w]
                        tl = actp.tile([P, AT], bf16, tag="lt")
                        nc.sync.dma_start(tl[:, :w], src)
                        nc.scalar.activation(
                            tl[:, :w], tl[:, :w], Exp,
                            accum_out=acc[:, col:col + 1])
                        col += 1
                        off += w
                    elif kind == "D":
                        src = logits[b * P:(b + 1) * P, off:off + w]
                        x = dvein.tile([P, DH], bf16, tag="dx")
                        nc.sync.dma_start(x[:], src)
                        if dve_half(x[:], P, col):
                            col += 1
                        off += w
                    elif kind == "X":
                        assert ei_prev is None
                        src = logits[b * P:b * P + XR, VM:V]
                        tl = xyp.tile([XR, D2], bf16, tag="xt")
                        nc.sync.dma_start(tl[:], src)
                        nc.scalar.activation(
                            tl[:], tl[:], Exp,
                            accum_out=acc[0:XR, col:col + 1])
                        col += 1
                    else:  # "Y"
                        assert ei_prev is None
                        src = logits[b * P + XR:(b + 1) * P, VM:V]
                        ty = xyp.tile([YR, D2], bf16, tag="xt")
                        nc.sync.dma_start(ty[:], src)
                        for c in range(D2 // DH):
                            if dve_half(ty[:, c * DH:(c + 1) * DH], YR, col):
                                col += 1
                assert off == VM and ei_prev is None
                assert col - c0 == _BLK_NCOLS[b]
                nc.sync.dma_start(out[:, c0:col], acc[:, c0:col])

    nc.compile()
    return nc


def _get_program():
    global _PROGRAM
    if _PROGRAM is None:
        _PROGRAM = _build_program()
    return _PROGRAM


def _make_in_maps(logits, ref_logprobs, labels):
    import ml_dtypes

    lg = np.asarray(logits).reshape(T, V)
    if lg.dtype != ml_dtypes.bfloat16:
        lg = lg.astype(ml_dtypes.bfloat16)
    valid = (np.asarray(labels).reshape(T) != IGNORE_INDEX)
    in_maps = [{"logits": np.ascontiguousarray(lg[c * TS:(c + 1) * TS])}
               for c in range(N_CORES)]
    return in_maps, float(valid.sum())


def _run(in_maps, trace=False, **kw):
    from concourse.bass_utils import run_bass_kernel_spmd

    nc = _get_program()
    return run_bass_kernel_spmd(nc, in_maps, list(range(N_CORES)),
                                trace=trace, **kw)


def kernel(logits, ref_logprobs, labels):
    import ml_dtypes

    lg = np.asarray(logits).reshape(T, V)
    if lg.dtype != ml_dtypes.bfloat16:
        lg = lg.astype(ml_dtypes.bfloat16)
    rl = np.asarray(ref_logprobs, dtype=np.float32).reshape(T)
    lb = np.asarray(labels).reshape(T).astype(np.int64)

    in_maps, count = _make_in_maps(lg, rl, lb)
    res = _run(in_maps)

    # per-token sumexp from the raw accumulator columns.
    # "F" col c: token c*256 + b*128 + p      (rows 0..127)
    # "X" col:   token c*256 + b*128 + p      (rows 0..91)
    # "Y" col:   token c*256 + b*128 + 92 + p (rows 0..35)
    S = np.zeros(T, np.float64)
    for c in range(N_CORES):
        o = np.asarray(res.results[c]["out"], dtype=np.float64)
        col = 0
        for bk in range(NB):
            base = c * TS + bk * P
            for kind, rows in _BLK_COLS[bk]:
                if kind == "Y":
                    S[base + XR:base + XR + rows] += o[:rows, col]
                else:
                    S[base:base + rows] += o[:rows, col]
                col += 1

    valid = lb != IGNORE_INDEX
    idx = np.clip(lb, 0, V - 1)
    lab = lg[np.arange(T), idx].astype(np.float64)
    loss = np.where(valid, np.log(S) - lab, 0.0)
    ratio = np.where(valid, np.exp(lab - rl.astype(np.float64)) / S, 1.0)
    total = float((loss * ratio).sum())
    return np.float32(total / count)


# revision 4
# speedup vs baseline: 1.0884x; 1.0884x over previous
"""Trainium2 Bass kernel for CE-with-importance-ratio loss.

Reference computation (B=1, T=2048, V=128256):
    logp = log_softmax(logits.f32, axis=-1)
    sel  = logp[t, labels[t]]
    loss = -sel                 (0 where label == -100)
    ratio = exp(sel - ref_logprobs)   (1 where ignored)
    out = sum(loss * ratio) / count_valid

Sharding: token-parallel across 8 NeuronCores (256 tokens/core).
Each core streams its [256, 128256] bf16 logit shard once from HBM
(tokens on partitions, vocab on the free axis) and emits ONLY the
per-token sum(exp(x)) ([128, 2] f32); all O(T) finishing math
(label-logit gather, ln, importance ratio, masking, reduction) runs
on the host.  The kernel is DMA-bound; the two compute engines split
each 128-token block's vocab sweep with ~30% slack each:

  ScalarE: exact exp+accumulate (1 elem/lane/cycle) on 64128 cols
           per block (tiles of <=16032).
  VectorE: Schraudolph bit-trick exp on 64128 cols per block
           (32 halves of 2004): i16 = trunc(x * 2^7*log2e + B),
           computed by one tensor_scalar (bf16 in, 2x mode), then one
           scalar_tensor_tensor that bitcasts the two 2004-col halves
           to bf16, adds them, and free-dim-accumulates (~1.12 ns/col
           for the pair).  B is bias-calibrated for bf16 N(0,1)
           logits (residual bias ~1e-5; loss error ~1e-6).

SDMA engine balance: walrus splits each [n, w] HWDGE DMA into
d = (largest divisor of n <= 16) equal line-chunks handed to SDMA
engines E0..E(d-1) in order, so a [128, w] tile gives every engine 8
lines.  HW traces show engine 15 runs ~15% slower per byte than
engines 0-14 (uniformly across packet sizes), so a uniform stream is
paced by E15 (99% busy; others 85%): 195.5us vs the ~183.5us
HBM-per-NC floor.  Fix: issue 8 of the 32 DVE chunks per block as a
[120, w] + [8, w] dma_start pair into the same tile.  The [120, w]
part fans to 15 engines (E15 gets nothing), the [8, w] part to
E0..E7, so per split chunk E15 sheds its 8 lines (32KB) while E0-7
gain one line each.  Per block E15 then carries 8 x 112224 x 2B =
1.80MB (~85-91us at its 20-21GB/s) ~= the HBM-floor half-stream,
with E0-7 at ~2.08MB (~85us at 24.6GB/s).  Compute, SBUF layout and
host math are untouched by the split.

Block 1's stream ends with small ACT tiles (8016/2004/2004) so the
compute tail after the final DMA byte is ~2-3 us.  No Ln on device,
no indirect gather, no matmul.
"""

import numpy as np

P = 128
B, T, V = 1, 2048, 128256
N_CORES = 8
TS = T // N_CORES          # tokens per core (256)
NB = TS // P               # token blocks per core (2)
IGNORE_INDEX = -100

AT = 16032                 # ScalarE tile width
DU = 4008                  # VectorE unit width (two 2004 halves)
DH = DU // 2
SPLIT_ROWS = 120           # [120,w]+[8,w] split: E15 carries no bytes

# Schraudolph exp, 16-bit flavor: i16 = trunc(x * A + B); bitcast the
# i16 to bf16 ~= exp(x).  A = 2^7 * log2(e).  B = 127*2^7 minus a
# bias-correction calibrated on bf16-rounded N(0,1) samples (kills the
# +3.8% mean sawtooth bias of the classic constant; residual bias ~1e-5,
# per-token noise ~1.1e-4 RMS).  The DVE's f32->i16 convert rounds to
# nearest (measured on HW: a +0.5 LSB shift vs the truncating numpy
# model appeared as a 1.22e-3 loss error), so the truncation-calibrated
# 16249.1316 is shifted down by 0.5.
# The 16-bit datapath keeps every DVE op in a packed perf mode.
_A_CONST = 184.6649652337873
_B_CONST = 16248.6316

# Per-block stream layout: (kind, width) in DMA issue order.  DVE chunks
# are DH wide and pair up: consecutive chunks share one accumulate.
# "S" chunks are DVE chunks DMA'd as a [120,w]+[8,w] pair (engine-15
# offload); 8 per block, spread evenly through the stream.
_B0_LAYOUT = []
for _ in range(4):
    _B0_LAYOUT += [("A", AT)] + [("D", DH), ("S", DH), ("D", DH), ("D", DH),
                                 ("D", DH), ("S", DH), ("D", DH), ("D", DH)]
# Final block: the last ~20 us of stream interleaves small ACT tiles
# with DVE chunks so neither engine is left with a backlog when the
# final DMA byte lands.
_B1_LAYOUT = []
for _ in range(3):
    _B1_LAYOUT += [("A", AT)] + [("D", DH), ("S", DH), ("D", DH), ("D", DH),
                                 ("D", DH), ("S", DH), ("D", DH), ("D", DH)]
# Tail pacing: ScalarE needs ~2.7us per small tile (ACTIVATE+ACC_READ)
# and DVE ~4.3us per chunk pair, so the last items are spaced such that
# each engine is drained when its final input lands: only one A2004
# after the last byte, last D pair ~2.4us of stream before the end.
_B1_LAYOUT += [("A", 8016), ("S", DH), ("S", DH), ("D", DH),
               ("A", 4008), ("D", DH), ("D", DH), ("D", DH),
               ("A", 2004), ("D", DH), ("D", DH),
               ("A", 2004)]
_LAYOUTS = [_B0_LAYOUT, _B1_LAYOUT]
for _l in _LAYOUTS:
    assert sum(w for _, w in _l) == V
    assert sum(1 for k, _ in _l if k in ("D", "S")) % 2 == 0
    assert sum(1 for k, _ in _l if k == "S") == 8
# accumulator columns per block: one per A tile, one per D chunk pair
_BLK_NCOLS = [sum(1 for k, _ in _l if k == "A")
              + sum(1 for k, _ in _l if k in ("D", "S")) // 2
              for _l in _LAYOUTS]
_NCOLS = sum(_BLK_NCOLS)

_PROGRAM = None


def _build_program():
    import concourse.bacc as bacc
    import concourse.mybir as mybir
    import concourse.tile as tile

    f32 = mybir.dt.float32
    bf16 = mybir.dt.bfloat16
    i16 = mybir.dt.int16

    nc = bacc.Bacc("TRN2", target_bir_lowering=False, debug=False,
                   num_devices=N_CORES)

    logits = nc.dram_tensor("logits", [TS, V], bf16, kind="ExternalInput").ap()
    # Raw accumulator columns per block; the host does the column sum.
    # Block 0's half is DMA'd out mid-kernel (hidden under the stream),
    # so the post-stream critical path is just accum-read -> out issue.
    out = nc.dram_tensor("out", [P, _NCOLS], f32, kind="ExternalOutput").ap()

    Exp = mybir.ActivationFunctionType.Exp
    Add, Mul = mybir.AluOpType.add, mybir.AluOpType.mult

    with tile.TileContext(nc) as tc:
        with (
            tc.tile_pool(name="small", bufs=1) as small,
            tc.tile_pool(name="act", bufs=4) as actp,
            tc.tile_pool(name="dvein", bufs=10) as dvein,
            tc.tile_pool(name="dvet", bufs=4) as dvet,
            tc.tile_pool(name="dveval", bufs=2) as dveval,
        ):
            acc = small.tile([P, _NCOLS], f32)

            col = 0
            for b, layout in enumerate(_LAYOUTS):
                c0 = col
                off = 0
                ei_prev = None
                for kind, w in layout:
                    src = logits[b * P:(b + 1) * P, off:off + w]
                    if kind == "A":
                        tl = actp.tile([P, AT], bf16, tag="lt")
                        nc.sync.dma_start(tl[:, :w], src)
                        nc.scalar.activation(
                            tl[:, :w], tl[:, :w], Exp,
                            accum_out=acc[:, col:col + 1])
                        col += 1
                    else:
                        x = dvein.tile([P, DH], bf16, tag="dx")
                        if kind == "S":
                            nc.sync.dma_start(
                                x[:SPLIT_ROWS], src[:SPLIT_ROWS])
                            nc.sync.dma_start(
                                x[SPLIT_ROWS:], src[SPLIT_ROWS:])
                        else:
                            nc.sync.dma_start(x[:], src)
                        ei = dvet.tile([P, DH], i16, tag="ei")
                        nc.vector.tensor_scalar(
                            ei[:], x[:], _A_CONST, _B_CONST, Mul, Add)
                        if ei_prev is None:
                            ei_prev = ei
                        else:
                            val = dveval.tile([P, DH], bf16, tag="val")
                            nc.vector.scalar_tensor_tensor(
                                val[:], ei_prev[:].bitcast(bf16), 1.0,
                                ei[:].bitcast(bf16), Mul, Add,
                                accum_out=acc[:, col:col + 1])
                            ei_prev = None
                            col += 1
                    off += w
                assert off == V and ei_prev is None
                assert col - c0 == _BLK_NCOLS[b]
                nc.sync.dma_start(out[:, c0:col], acc[:, c0:col])

    nc.compile()
    return nc


def _get_program():
    global _PROGRAM
    if _PROGRAM is None:
        _PROGRAM = _build_program()
    return _PROGRAM


def _make_in_maps(logits, ref_logprobs, labels):
    import ml_dtypes

    lg = np.asarray(logits).reshape(T, V)
    if lg.dtype != ml_dtypes.bfloat16:
        lg = lg.astype(ml_dtypes.bfloat16)
    valid = (np.asarray(labels).reshape(T) != IGNORE_INDEX)
    in_maps = [{"logits": np.ascontiguousarray(lg[c * TS:(c + 1) * TS])}
               for c in range(N_CORES)]
    return in_maps, float(valid.sum())


def _run(in_maps, trace=False, **kw):
    from concourse.bass_utils import run_bass_kernel_spmd

    nc = _get_program()
    return run_bass_kernel_spmd(nc, in_maps, list(range(N_CORES)),
                                trace=trace, **kw)


def kernel(logits, ref_logprobs, labels):
    import ml_dtypes

    lg = np.asarray(logits).reshape(T, V)
    if lg.dtype != ml_dtypes.bfloat16:
        lg = lg.astype(ml_dtypes.bfloat16)
    rl = np.asarray(ref_logprobs, dtype=np.float32).reshape(T)
    lb = np.asarray(labels).reshape(T).astype(np.int64)

    in_maps, count = _make_in_maps(lg, rl, lb)
    res = _run(in_maps)

    # per-token sumexp: raw accumulator columns, summed per block on host.
    # out[p, c0:c1] belongs to token c*256 + b*128 + p.
    S = np.empty(T, np.float64)
    for c in range(N_CORES):
        o = np.asarray(res.results[c]["out"], dtype=np.float64)
        c0 = 0
        for b in range(NB):
            c1 = c0 + _BLK_NCOLS[b]
            S[c * TS + b * P:c * TS + (b + 1) * P] = o[:, c0:c1].sum(axis=1)
            c0 = c1
    valid = lb != IGNORE_INDEX
    idx = np.clip(lb, 0, V - 1)
    lab = lg[np.arange(T), idx].astype(np.float64)
    loss = np.where(valid, np.log(S) - lab, 0.0)
    ratio = np.where(valid, np.exp(lab - rl.astype(np.float64)) / S, 1.0)
    total = float((loss * ratio).sum())
    return np.float32(total / count)


# revision 5
# speedup vs baseline: 1.0953x; 1.0063x over previous
"""Trainium2 Bass kernel for CE-with-importance-ratio loss.

Reference computation (B=1, T=2048, V=128256):
    logp = log_softmax(logits.f32, axis=-1)
    sel  = logp[t, labels[t]]
    loss = -sel                 (0 where label == -100)
    ratio = exp(sel - ref_logprobs)   (1 where ignored)
    out = sum(loss * ratio) / count_valid

Sharding: token-parallel across 8 NeuronCores (256 tokens/core).
Each core streams its [256, 128256] bf16 logit shard once from HBM
(tokens on partitions, vocab on the free axis) and emits ONLY the
per-token sum(exp(x)) ([128, 2] f32); all O(T) finishing math
(label-logit gather, ln, importance ratio, masking, reduction) runs
on the host.  The kernel is DMA-bound; the two compute engines split
each 128-token block's vocab sweep with ~30% slack each:

  ScalarE: exact exp+accumulate (1 elem/lane/cycle) on 64128 cols
           per block (tiles of <=16032).
  VectorE: Schraudolph bit-trick exp on 64128 cols per block
           (32 halves of 2004): i16 = trunc(x * 2^7*log2e + B),
           computed by one tensor_scalar (bf16 in, 2x mode), then one
           scalar_tensor_tensor that bitcasts the two 2004-col halves
           to bf16, adds them, and free-dim-accumulates (~1.12 ns/col
           for the pair).  B is bias-calibrated for bf16 N(0,1)
           logits (residual bias ~1e-5; loss error ~1e-6).

SDMA engine balance: walrus splits each [n, w] HWDGE DMA into
d = (largest divisor of n <= 16) equal line-chunks handed to SDMA
engines E0..E(d-1) in order, so a [128, w] tile gives every engine 8
lines.  HW traces show engine 15 runs ~15% slower per byte than
engines 0-14 (uniformly across packet sizes), so a uniform stream is
paced by E15 (99% busy; others 85%): 195.5us vs the ~183.5us
HBM-per-NC floor.  Fix: issue 8 of the 32 DVE chunks per block as a
[120, w] + [8, w] dma_start pair into the same tile.  The [120, w]
part fans to 15 engines (E15 gets nothing), the [8, w] part to
E0..E7, so per split chunk E15 sheds its 8 lines (32KB) while E0-7
gain one line each.  Per block E15 then carries 8 x 112224 x 2B =
1.80MB (~85-91us at its 20-21GB/s) ~= the HBM-floor half-stream,
with E0-7 at ~2.08MB (~85us at 24.6GB/s).  Compute, SBUF layout and
host math are untouched by the split.

Block 1's stream ends with small ACT tiles (8016/2004/2004) so the
compute tail after the final DMA byte is ~2-3 us.  No Ln on device,
no indirect gather, no matmul.
"""

import numpy as np

P = 128
B, T, V = 1, 2048, 128256
N_CORES = 8
TS = T // N_CORES          # tokens per core (256)
NB = TS // P               # token blocks per core (2)
IGNORE_INDEX = -100

AT = 16032                 # ScalarE tile width
DU = 4008                  # VectorE unit width (two 2004 halves)
DH = DU // 2
SPLIT_ROWS = 120           # [120,w]+[8,w] split: E15 carries no bytes

# Schraudolph exp, 16-bit flavor: i16 = trunc(x * A + B); bitcast the
# i16 to bf16 ~= exp(x).  A = 2^7 * log2(e).  B = 127*2^7 minus a
# bias-correction calibrated on bf16-rounded N(0,1) samples (kills the
# +3.8% mean sawtooth bias of the classic constant; residual bias ~1e-5,
# per-token noise ~1.1e-4 RMS).  The DVE's f32->i16 convert rounds to
# nearest (measured on HW: a +0.5 LSB shift vs the truncating numpy
# model appeared as a 1.22e-3 loss error), so the truncation-calibrated
# 16249.1316 is shifted down by 0.5.
# The 16-bit datapath keeps every DVE op in a packed perf mode.
_A_CONST = 184.6649652337873
_B_CONST = 16248.6316

# Per-block stream layout: (kind, width) in DMA issue order.  DVE chunks
# are DH wide and pair up: consecutive chunks share one accumulate.
# "S" chunks are DVE chunks DMA'd as a [120,w]+[8,w] pair (engine-15
# offload); 8 per block, spread evenly through the stream.
_B0_LAYOUT = []
for _ in range(4):
    _B0_LAYOUT += [("A", AT)] + [("D", DH), ("S", DH), ("D", DH), ("D", DH),
                                 ("D", DH), ("S", DH), ("D", DH), ("D", DH)]
# Final block: the last ~20 us of stream interleaves small ACT tiles
# with DVE chunks so neither engine is left with a backlog when the
# final DMA byte lands.
_B1_LAYOUT = []
for _ in range(3):
    _B1_LAYOUT += [("A", AT)] + [("D", DH), ("S", DH), ("D", DH), ("D", DH),
                                 ("D", DH), ("S", DH), ("D", DH), ("D", DH)]
# Tail pacing: ScalarE needs ~2.7us per small tile (ACTIVATE+ACC_READ)
# and DVE ~4.3us per chunk pair, so the last items are spaced such that
# each engine is drained when its final input lands: only one A2004
# after the last byte, last D pair ~2.4us of stream before the end.
_B1_LAYOUT += [("A", 8016), ("S", DH), ("S", DH), ("D", DH),
               ("A", 4008), ("D", DH), ("D", DH), ("D", DH),
               ("A", 2004), ("D", DH), ("D", DH),
               ("A", 2004)]
_LAYOUTS = [_B0_LAYOUT, _B1_LAYOUT]
for _l in _LAYOUTS:
    assert sum(w for _, w in _l) == V
    assert sum(1 for k, _ in _l if k in ("D", "S")) % 2 == 0
    assert sum(1 for k, _ in _l if k == "S") == 8
# accumulator columns per block: one per A tile, one per D chunk pair
_BLK_NCOLS = [sum(1 for k, _ in _l if k == "A")
              + sum(1 for k, _ in _l if k in ("D", "S")) // 2
              for _l in _LAYOUTS]
_NCOLS = sum(_BLK_NCOLS)

_PROGRAM = None


def _patch_walrus_args():
    """Append --max-sem-num to the walrus (BIR->NEFF) compile flags.

    The NEFF epilogue individually zeroes every semaphore in the file
    ([3, max_sem_num)) split across the 5 engines — ~6.4us of the ~9.5us
    post-stream fixed cost at the default 256.  The kernel itself only
    uses sems up to ~165 (walrus reserves 0-149, bass allocates lazily
    from 150), so capping the file shortens the reset storm without
    touching behavior.
    """
    import concourse.bass_utils as bu

    if getattr(bu.get_walrus_args, "_sem_patched", False):
        return
    orig = bu.get_walrus_args

    def patched(*a, **kw):
        return orig(*a, **kw) + ["--max-sem-num=176"]

    patched._sem_patched = True
    bu.get_walrus_args = patched


def _build_program():
    import concourse.bacc as bacc
    import concourse.mybir as mybir
    import concourse.tile as tile

    _patch_walrus_args()

    f32 = mybir.dt.float32
    bf16 = mybir.dt.bfloat16
    i16 = mybir.dt.int16

    nc = bacc.Bacc("TRN2", target_bir_lowering=False, debug=False,
                   num_devices=N_CORES)

    logits = nc.dram_tensor("logits", [TS, V], bf16, kind="ExternalInput").ap()
    # Raw accumulator columns per block; the host does the column sum.
    # Block 0's half is DMA'd out mid-kernel (hidden under the stream),
    # so the post-stream critical path is just accum-read -> out issue.
    out = nc.dram_tensor("out", [P, _NCOLS], f32, kind="ExternalOutput").ap()

    Exp = mybir.ActivationFunctionType.Exp
    Add, Mul = mybir.AluOpType.add, mybir.AluOpType.mult

    with tile.TileContext(nc) as tc:
        with (
            tc.tile_pool(name="small", bufs=1) as small,
            tc.tile_pool(name="act", bufs=4) as actp,
            tc.tile_pool(name="dvein", bufs=10) as dvein,
            tc.tile_pool(name="dvet", bufs=4) as dvet,
            tc.tile_pool(name="dveval", bufs=2) as dveval,
        ):
            acc = small.tile([P, _NCOLS], f32)

            col = 0
            for b, layout in enumerate(_LAYOUTS):
                c0 = col
                off = 0
                ei_prev = None
                for kind, w in layout:
                    src = logits[b * P:(b + 1) * P, off:off + w]
                    if kind == "A":
                        tl = actp.tile([P, AT], bf16, tag="lt")
                        nc.sync.dma_start(tl[:, :w], src)
                        nc.scalar.activation(
                            tl[:, :w], tl[:, :w], Exp,
                            accum_out=acc[:, col:col + 1])
                        col += 1
                    else:
                        x = dvein.tile([P, DH], bf16, tag="dx")
                        if kind == "S":
                            nc.sync.dma_start(
                                x[:SPLIT_ROWS], src[:SPLIT_ROWS])
                            nc.sync.dma_start(
                                x[SPLIT_ROWS:], src[SPLIT_ROWS:])
                        else:
                            nc.sync.dma_start(x[:], src)
                        ei = dvet.tile([P, DH], i16, tag="ei")
                        nc.vector.tensor_scalar(
                            ei[:], x[:], _A_CONST, _B_CONST, Mul, Add)
                        if ei_prev is None:
                            ei_prev = ei
                        else:
                            val = dveval.tile([P, DH], bf16, tag="val")
                            nc.vector.scalar_tensor_tensor(
                                val[:], ei_prev[:].bitcast(bf16), 1.0,
                                ei[:].bitcast(bf16), Mul, Add,
                                accum_out=acc[:, col:col + 1])
                            ei_prev = None
                            col += 1
                    off += w
                assert off == V and ei_prev is None
                assert col - c0 == _BLK_NCOLS[b]
                nc.sync.dma_start(out[:, c0:col], acc[:, c0:col])

    nc.compile()
    return nc


def _get_program():
    global _PROGRAM
    if _PROGRAM is None:
        _PROGRAM = _build_program()
    return _PROGRAM


def _make_in_maps(logits, ref_logprobs, labels):
    import ml_dtypes

    lg = np.asarray(logits).reshape(T, V)
    if lg.dtype != ml_dtypes.bfloat16:
        lg = lg.astype(ml_dtypes.bfloat16)
    valid = (np.asarray(labels).reshape(T) != IGNORE_INDEX)
    in_maps = [{"logits": np.ascontiguousarray(lg[c * TS:(c + 1) * TS])}
               for c in range(N_CORES)]
    return in_maps, float(valid.sum())


def _run(in_maps, trace=False, **kw):
    from concourse.bass_utils import run_bass_kernel_spmd

    nc = _get_program()
    return run_bass_kernel_spmd(nc, in_maps, list(range(N_CORES)),
                                trace=trace, **kw)


def kernel(logits, ref_logprobs, labels):
    import ml_dtypes

    lg = np.asarray(logits).reshape(T, V)
    if lg.dtype != ml_dtypes.bfloat16:
        lg = lg.astype(ml_dtypes.bfloat16)
    rl = np.asarray(ref_logprobs, dtype=np.float32).reshape(T)
    lb = np.asarray(labels).reshape(T).astype(np.int64)

    in_maps, count = _make_in_maps(lg, rl, lb)
    res = _run(in_maps)

    # per-token sumexp: raw accumulator columns, summed per block on host.
    # out[p, c0:c1] belongs to token c*256 + b*128 + p.
    S = np.empty(T, np.float64)
    for c in range(N_CORES):
        o = np.asarray(res.results[c]["out"], dtype=np.float64)
        c0 = 0
        for b in range(NB):
            c1 = c0 + _BLK_NCOLS[b]
            S[c * TS + b * P:c * TS + (b + 1) * P] = o[:, c0:c1].sum(axis=1)
            c0 = c1
    valid = lb != IGNORE_INDEX
    idx = np.clip(lb, 0, V - 1)
    lab = lg[np.arange(T), idx].astype(np.float64)
    loss = np.where(valid, np.log(S) - lab, 0.0)
    ratio = np.where(valid, np.exp(lab - rl.astype(np.float64)) / S, 1.0)
    total = float((loss * ratio).sum())
    return np.float32(total / count)
